# revision 5
# baseline (speedup 1.0000x reference)
"""Trainium2 Bass kernel for the sparse-MoE block (top-2 of 8 experts).

Strategy: the router (a tiny [T,H]x[H,E] matmul + top-2) and the token
dispatch run on the host; the expert FFNs -- 99.97% of the FLOPs -- run on
8 NeuronCores. Sharding is F-parallel: each core holds a 512-wide slice of
the FFN intermediate dimension for ALL 8 experts, processes every expert's
gathered token group against its slice, and returns a partial down-proj
output. The host sums the 8 partials and scatter-adds into token order
with the routing weights. This is load-balanced regardless of routing.

Matmuls run in float32r (full fp32 data, full-rate PE mode) with fp32
PSUM accumulation.
"""

import os

import numpy as np

import concourse.bass as bass
import concourse.tile as tile
from concourse import mybir
from concourse.bass_utils import run_bass_kernel_spmd

# Set by the last kernel() call: the Bass program and results, so test.py
# can run the TimelineSim cost model on the exact program that executed.
LAST_RESULTS = None
LAST_NC = None

B, S, H, F, E = 2, 2048, 1024, 4096, 8
TOP_K = 2
NCORES = 8
FS = F // NCORES  # 512
F32R = mybir.dt.float32r
F32 = mybir.dt.float32
SILU = mybir.ActivationFunctionType.Silu
MULT = mybir.AluOpType.mult


def _split_multi_waits(nc, max_waits=1):
    """This toolchain's walrus codegen supports one sync-wait per
    instruction; Tile attaches as many as needed. Hoist extras onto
    standalone NoOps just before the instruction on the same engine
    (engine streams execute in order, so semantics are preserved)."""
    total = 0
    for f in nc.m.functions:
        for bb in f.blocks:
            new_insts = []
            changed = False
            for inst in bb.instructions:
                si = inst.sync_info
                waits = list(si.on_wait) if si and si.on_wait else []
                if len(waits) > max_waits:
                    for w in waits[:-max_waits]:
                        nop = mybir.InstNoOp(
                            name=nc.get_next_instruction_name(), ins=[], outs=[]
                        )
                        nop.engine = inst.engine
                        nop.sync_info = mybir.SyncInfo(on_wait=[w], on_update=[])
                        new_insts.append(nop)
                        total += 1
                    inst.sync_info = mybir.SyncInfo(
                        on_wait=waits[-max_waits:],
                        on_update=list(si.on_update) if si.on_update else [],
                    )
                    changed = True
                new_insts.append(inst)
            if changed:
                bb.instructions = new_insts
    return total


def _expert_chunk_widths(cnt):
    # Split a token count into chunk widths <=512, keeping every chunk
    # >=256 where possible (fp32r matmuls run at full rate only when the
    # moving free dim is >=256). No padding: widths sum to cnt exactly.
    if cnt == 0:
        return []
    if cnt <= 512:
        return [cnt]
    n512, tail = divmod(cnt, 512)
    if tail == 0:
        return [512] * n512
    if tail >= 256:
        return [512] * n512 + [tail]
    # borrow from the last full chunk: 512 + tail -> (256 + tail) + 256
    return [512] * (n512 - 1) + [256 + tail, 256]


def _make_chunks(pads):
    chunks = []
    base = 0
    for e, pad in enumerate(pads):
        off = 0
        for w in _expert_chunk_widths(pad):
            chunks.append((e, base + off, w))
            off += w
        base += pad
    return chunks, base


def _build_program(pads, loop_iters=1, bufs=None, xt_eng='scalar', y_eng='sync'):
    bufs = {**{'w': 2, 'x': 3, 'a': 3, 'g': 3, 'y': 3, 'pg': 1, 'pu': 5, 'py': 2}, **(bufs or {})}
    chunks, CT = _make_chunks(pads)
    nc = bass.Bass("TRN2", target_bir_lowering=False, debug=False, num_devices=NCORES)
    xt = nc.declare_dram_parameter("xt", [H, CT], F32R, isOutput=False)
    wg = nc.declare_dram_parameter("wg", [E, H, FS], F32R, isOutput=False)
    wu = nc.declare_dram_parameter("wu", [E, H, FS], F32R, isOutput=False)
    wd = nc.declare_dram_parameter("wd", [E, FS, H], F32R, isOutput=False)
    yp = nc.declare_dram_parameter("yp", [CT, H], F32, isOutput=True)

    xt3 = xt[:].rearrange("(ko p) c -> p ko c", p=128)  # [128, 8, CT]

    with tile.TileContext(nc) as tc:
        with (
            tc.tile_pool(name="wpool", bufs=bufs["w"]) as wpool,
            tc.tile_pool(name="xpool", bufs=bufs["x"]) as xpool,
            tc.tile_pool(name="apool", bufs=bufs["a"]) as apool,
            tc.tile_pool(name="gpool", bufs=bufs["g"]) as gpool,
            tc.tile_pool(name="ypool", bufs=bufs["y"]) as ypool,
            tc.tile_pool(name="pga", bufs=bufs["pg"], space="PSUM") as pg_pool,
            tc.tile_pool(name="pua", bufs=bufs["pu"], space="PSUM") as pu_pool,
            tc.tile_pool(name="pyb", bufs=bufs["py"], space="PSUM") as py_pool,
        ):
            import contextlib

            loop_ctx = (
                tc.For_i(0, loop_iters) if loop_iters > 1 else contextlib.nullcontext()
            )

            def load_weights(e):
                wgt = wpool.tile([128, H // 128, FS], F32R, tag="wg")
                wut = wpool.tile([128, H // 128, FS], F32R, tag="wu")
                wdt = wpool.tile([128, FS // 128, H], F32R, tag="wd")
                nc.sync.dma_start(wgt[:], wg[e].rearrange("(ko p) f -> p ko f", p=128))
                nc.sync.dma_start(wut[:], wu[e].rearrange("(ko p) f -> p ko f", p=128))
                nc.sync.dma_start(wdt[:], wd[e].rearrange("(ko p) h -> p ko h", p=128))
                return wgt, wut, wdt

            def stage_b(act, w, c0, wdt):
                for cs in range(-(-w // 128)):
                    m = min(128, w - cs * 128)
                    yt = ypool.tile([128, H], F32, tag="y")
                    for ht in range(2):
                        py = py_pool.tile([128, 512], F32, tag="py")
                        for kf in range(FS // 128):
                            nc.tensor.matmul(
                                py[:m],
                                act[:, kf, cs * 128 : cs * 128 + m],
                                wdt[:, kf, ht * 512 : (ht + 1) * 512],
                                start=(kf == 0),
                                stop=(kf == FS // 128 - 1),
                            )
                        nc.vector.tensor_copy(yt[:m, ht * 512 : (ht + 1) * 512], py[:m])
                    getattr(nc, y_eng).dma_start(
                        yp[c0 + cs * 128 : c0 + cs * 128 + m, :], yt[:m]
                    )

            with loop_ctx:
                cur_e = -1
                wgt = wut = wdt = None
                prev = None
                for e, c0, w in chunks:
                    xtile = xpool.tile([128, H // 128, 512], F32R, tag="xt")
                    getattr(nc, xt_eng).dma_start(xtile[:, :, :w], xt3[:, :, c0 : c0 + w])
                    if e != cur_e:
                        wgt, wut, wdt = load_weights(e)
                        cur_e = e
                    act = apool.tile([128, FS // 128, 512], F32R, tag="act")
                    for ft in range(FS // 128):
                        pg = pg_pool.tile([128, 512], F32, tag="pg")
                        pu = pu_pool.tile([128, 512], F32, tag="pu")
                        for k in range(H // 128):
                            nc.tensor.matmul(
                                pg[:, :w],
                                wgt[:, k, ft * 128 : (ft + 1) * 128],
                                xtile[:, k, :w],
                                start=(k == 0),
                                stop=(k == H // 128 - 1),
                            )
                        for k in range(H // 128):
                            nc.tensor.matmul(
                                pu[:, :w],
                                wut[:, k, ft * 128 : (ft + 1) * 128],
                                xtile[:, k, :w],
                                start=(k == 0),
                                stop=(k == H // 128 - 1),
                            )
                        gs = gpool.tile([128, 512], F32, tag="g")
                        nc.scalar.activation(gs[:, :w], pg[:, :w], SILU)
                        nc.vector.tensor_tensor(act[:, ft, :w], gs[:, :w], pu[:, :w], MULT)
                    if prev is not None:
                        stage_b(*prev)
                    prev = (act, w, c0, wdt)
                stage_b(*prev)

    _split_multi_waits(nc)
    return nc, CT


_program_cache = {}


def _get_program(pads):
    key = tuple(pads)
    if key not in _program_cache:
        _program_cache[key] = _build_program(pads)
    return _program_cache[key]


def _route(x, w_gate):
    """Host router: softmax(fp32) then top-2, matching jax.lax.top_k
    tie-breaking (lowest index first)."""
    logits = x @ w_gate  # [T, E] fp32
    m = logits.max(axis=-1, keepdims=True)
    p = np.exp(logits - m, dtype=np.float32)
    p /= p.sum(axis=-1, keepdims=True)
    order = np.argsort(-p, axis=-1, kind="stable")
    sel = order[:, :TOP_K]
    rw = np.take_along_axis(p, sel, axis=-1).astype(np.float32)
    return sel, rw


def kernel(hidden_states, w_gate, w_gate_proj, w_up_proj, w_down_proj):
    x = np.asarray(hidden_states, dtype=np.float32).reshape(-1, H)
    w_gate = np.asarray(w_gate, dtype=np.float32)
    WG = np.asarray(w_gate_proj, dtype=np.float32)
    WU = np.asarray(w_up_proj, dtype=np.float32)
    WD = np.asarray(w_down_proj, dtype=np.float32)
    T = x.shape[0]

    sel, rw = _route(x, w_gate)

    idx, wtok, cnts = [], [], []
    for e in range(E):
        mask0 = sel[:, 0] == e
        mask1 = sel[:, 1] == e
        ie = np.nonzero(mask0 | mask1)[0]
        idx.append(ie)
        wtok.append(np.where(mask0[ie], rw[ie, 0], rw[ie, 1]).astype(np.float32))
        cnts.append(len(ie))

    # fp32r matmuls require an even moving free dim; round each expert's
    # token count up to even (the pad column is zeros).
    ecnts = [c + (c & 1) for c in cnts]
    nc, CT = _get_program(ecnts)

    base = np.concatenate([[0], np.cumsum(ecnts)])
    xt = np.zeros((H, CT), dtype=np.float32)
    for e in range(E):
        if cnts[e]:
            xt[:, base[e] : base[e] + cnts[e]] = x[idx[e]].T

    in_maps = []
    for c in range(NCORES):
        in_maps.append(
            {
                "xt": xt,
                "wg": np.ascontiguousarray(WG[:, :, c * FS : (c + 1) * FS]),
                "wu": np.ascontiguousarray(WU[:, :, c * FS : (c + 1) * FS]),
                "wd": np.ascontiguousarray(WD[:, c * FS : (c + 1) * FS, :]),
            }
        )
    global LAST_RESULTS, LAST_NC
    res = run_bass_kernel_spmd(nc, in_maps, list(range(NCORES)))
    LAST_RESULTS = res
    LAST_NC = nc

    ysum = res.results[0]["yp"]
    for i in range(1, NCORES):
        ysum = ysum + res.results[i]["yp"]

    out = np.zeros((T, H), dtype=np.float32)
    for e in range(E):
        if cnts[e]:
            out[idx[e]] += ysum[base[e] : base[e] + cnts[e]] * wtok[e][:, None]
    return out.reshape(B, S, H).astype(np.float32)



# revision 34
# speedup vs baseline: 1.0648x; 1.0648x over previous
"""Trainium2 Bass kernel for the sparse-MoE block (top-2 of 8 experts).

Strategy: the router (a tiny [T,H]x[H,E] matmul + top-2) and the token
dispatch run on the host; the expert FFNs -- 99.97% of the FLOPs -- run on
8 NeuronCores. Sharding is F-parallel: each core holds a 512-wide slice of
the FFN intermediate dimension for ALL 8 experts, processes every expert's
gathered token group against its slice, and returns a partial down-proj
output (transposed, [H, T]). The host sums the 8 partials and scatter-adds
into token order with the routing weights. This is load-balanced regardless
of routing.

All matmul operands are bf16 (same full PE rate as fp32r, half the HBM
traffic); accumulation stays fp32 in PSUM and the partial outputs are
written back fp32 straight from PSUM. Tokens are always the moving dim
(gate/up AND down-proj), so ragged expert token counts waste no PE cycles.
"""

import os

import numpy as np

import concourse.bass as bass
import concourse.tile as tile
from concourse import mybir
from concourse.bass_utils import run_bass_kernel_spmd

# Set by the last kernel() call: the Bass program and results, so test.py
# can run the TimelineSim cost model on the exact program that executed.
LAST_RESULTS = None
LAST_NC = None

B, S, H, F, E = 2, 2048, 1024, 4096, 8
TOP_K = 2
NCORES = 8
FS = F // NCORES  # 512
KO = H // 128  # 8
KF = FS // 128  # 4
HT = H // 128  # 8
BF16 = mybir.dt.bfloat16
F32 = mybir.dt.float32
SILU = mybir.ActivationFunctionType.Silu
MULT = mybir.AluOpType.mult


def _split_multi_waits(nc, max_waits=1):
    """This toolchain's walrus codegen supports one sync-wait per
    instruction; Tile attaches as many as needed. Hoist extras onto
    standalone NoOps just before the instruction on the same engine
    (engine streams execute in order, so semantics are preserved)."""
    total = 0
    for f in nc.m.functions:
        for bb in f.blocks:
            new_insts = []
            changed = False
            for inst in bb.instructions:
                si = inst.sync_info
                waits = list(si.on_wait) if si and si.on_wait else []
                if len(waits) > max_waits:
                    for w in waits[:-max_waits]:
                        nop = mybir.InstNoOp(
                            name=nc.get_next_instruction_name(), ins=[], outs=[]
                        )
                        nop.engine = inst.engine
                        nop.sync_info = mybir.SyncInfo(on_wait=[w], on_update=[])
                        new_insts.append(nop)
                        total += 1
                    inst.sync_info = mybir.SyncInfo(
                        on_wait=waits[-max_waits:],
                        on_update=list(si.on_update) if si.on_update else [],
                    )
                    changed = True
                new_insts.append(inst)
            if changed:
                bb.instructions = new_insts
    return total


def _expert_chunk_widths(cnt):
    # Split a token count into chunk widths <=512. bf16 matmuls run at
    # 1 cycle/row for any moving size, so tails can be any even width.
    if cnt == 0:
        return []
    n512, tail = divmod(cnt, 512)
    return [512] * n512 + ([tail] if tail else [])


def _make_chunks(pads):
    # Emit experts so the one with the smallest tail chunk comes last --
    # the final chunk's down-proj + y writeback is the drain tail of the
    # whole program.
    E_ = len(pads)
    base_of = np.concatenate([[0], np.cumsum(pads)])

    def tailw(e):
        ws = _expert_chunk_widths(pads[e])
        return ws[-1] if ws else 10**9

    order = sorted(range(E_), key=lambda e: (-tailw(e), e))
    chunks = []
    for e in order:
        off = 0
        for w in _expert_chunk_widths(pads[e]):
            chunks.append((e, int(base_of[e]) + off, w))
            off += w
    return chunks, int(base_of[-1])


def _build_program(pads, bufs=None):
    bufs = {**{"w": 3, "x": 3, "a": 3, "g": 4, "y": 3, "pg": 2, "pu": 3, "py": 3}, **(bufs or {})}
    chunks, CT = _make_chunks(pads)
    nc = bass.Bass("TRN2", target_bir_lowering=False, debug=False, num_devices=NCORES)
    xt = nc.declare_dram_parameter("xt", [H, CT], BF16, isOutput=False)
    wg = nc.declare_dram_parameter("wg", [E, H, FS], BF16, isOutput=False)
    wu = nc.declare_dram_parameter("wu", [E, H, FS], BF16, isOutput=False)
    wd = nc.declare_dram_parameter("wd", [E, FS, H], BF16, isOutput=False)
    yp = nc.declare_dram_parameter("yp", [H, CT], BF16, isOutput=True)

    xt3 = xt[:].rearrange("(ko p) c -> p ko c", p=128)  # [128, KO, CT]
    yp3 = yp[:].rearrange("(ht p) c -> p ht c", p=128)  # [128, HT, CT]

    with tile.TileContext(nc) as tc:
        with (
            tc.tile_pool(name="wpool", bufs=bufs["w"]) as wpool,
            tc.tile_pool(name="xpool", bufs=bufs["x"]) as xpool,
            tc.tile_pool(name="apool", bufs=bufs["a"]) as apool,
            tc.tile_pool(name="gpool", bufs=bufs["g"]) as gpool,
            tc.tile_pool(name="ypool", bufs=bufs["y"]) as ypool,
            tc.tile_pool(name="pga", bufs=bufs["pg"], space="PSUM") as pg_pool,
            tc.tile_pool(name="pua", bufs=bufs["pu"], space="PSUM") as pu_pool,
            tc.tile_pool(name="pyb", bufs=bufs["py"], space="PSUM") as py_pool,
        ):

            def load_weights(e, fine=False):
                # Piecewise loads keep individual transfers short so y
                # write-backs interleave on the DMA engines; `fine` (first
                # expert) quarters everything to shorten the pipeline fill.
                wgt = wpool.tile([128, KO, FS], BF16, tag="wg")
                wut = wpool.tile([128, KO, FS], BF16, tag="wu")
                wdt = wpool.tile([128, KF, H], BF16, tag="wd")
                wg3 = wg[e].rearrange("(ko p) f -> p ko f", p=128)
                wu3 = wu[e].rearrange("(ko p) f -> p ko f", p=128)
                wd3 = wd[e].rearrange("(ko p) h -> p ko h", p=128)
                gstep = KO // 4 if fine else KO // 2
                for k0 in range(0, KO, gstep):
                    nc.sync.dma_start(wgt[:, k0 : k0 + gstep], wg3[:, k0 : k0 + gstep])
                ustep = KO // 4 if fine else KO
                for k0 in range(0, KO, ustep):
                    nc.sync.dma_start(wut[:, k0 : k0 + ustep], wu3[:, k0 : k0 + ustep])
                dstep = KF // 4 if fine else KF
                for k0 in range(0, KF, dstep):
                    nc.sync.dma_start(wdt[:, k0 : k0 + dstep], wd3[:, k0 : k0 + dstep])
                return wgt, wut, wdt

            def load_x(c0, w, pieces=2):
                xtile = xpool.tile([128, KO, 512], BF16, tag="xt")
                step = KO // pieces
                for k0 in range(0, KO, step):
                    nc.scalar.dma_start(
                        xtile[:, k0 : k0 + step, :w], xt3[:, k0 : k0 + step, c0 : c0 + w]
                    )
                return xtile

            def stage_b(act, w, c0, wdt, ydma_step=HT):
                # Down-proj, tokens moving: py[ht] = sum_kf wdt[:,kf,ht]T @ act[:,kf,:]
                # The 8 ht strips collect into one SBUF tile and leave in a
                # single chunk-wide DMA: SP dispatch is 650ns per DMACopy, so
                # per-ht DMAs would serialize the end-of-program drain. (The
                # final chunks pass a smaller ydma_step so the last write-back
                # overlaps the trailing copies instead of following them.)
                ytile = ypool.tile([128, HT, 512], BF16, tag="y")
                for ht in range(HT):
                    py = py_pool.tile([128, 512], F32, tag="py")
                    for kf in range(KF):
                        nc.tensor.matmul(
                            py[:, :w],
                            wdt[:, kf, ht * 128 : (ht + 1) * 128],
                            act[:, kf, :w],
                            start=(kf == 0),
                            stop=(kf == KF - 1),
                        )
                    # Alternate the PSUM->SBUF copy between DVE and the Act
                    # engine so the per-chunk chain of 8 copies isn't
                    # serialized on one engine. (GPSIMD/Pool can't read PSUM.)
                    if ht % 2 == 0:
                        nc.vector.tensor_copy(ytile[:, ht, :w], py[:, :w])
                    else:
                        nc.scalar.activation(
                            ytile[:, ht, :w],
                            py[:, :w],
                            mybir.ActivationFunctionType.Copy,
                        )
                    if (ht + 1) % ydma_step == 0:
                        h0 = ht + 1 - ydma_step
                        nc.sync.dma_start(
                            yp3[:, h0 : ht + 1, c0 : c0 + w],
                            ytile[:, h0 : ht + 1, :w],
                        )

            # Expert order as emitted; prefetch the NEXT expert's weights as
            # soon as the current expert's first chunk is issued (wpool
            # bufs=3 keeps prev/cur/next weight sets resident).
            expert_seq = []
            for e, _, _ in chunks:
                if not expert_seq or expert_seq[-1] != e:
                    expert_seq.append(e)
            wtiles = {}
            nload = 0
            wtiles[expert_seq[0]] = load_weights(expert_seq[0], fine=True)
            nload = 1

            # x tiles are dispatched two chunks ahead: the Act sequencer only
            # reaches chunk j+2's dma_start after chunk j's silus, so a
            # 1-chunk lookahead arrives just-in-time and stalls the PE when
            # the intervening down-proj is a short tail chunk.
            xtiles = {}

            def issue_x(j):
                if j < len(chunks):
                    _, xc0, xw = chunks[j]
                    xtiles[j] = load_x(xc0, xw, pieces=4 if j == 0 else 2)

            issue_x(0)
            issue_x(1)

            cur_e = -1
            wgt = wut = wdt = None
            prev = None
            for j, (e, c0, w) in enumerate(chunks):
                new_expert = e != cur_e
                if new_expert:
                    wgt, wut, wdt = wtiles.pop(e)
                    cur_e = e
                xtile = xtiles.pop(j)
                act = apool.tile([128, KF, 512], BF16, tag="act")

                def gate_mm(ft, pg):
                    for k in range(KO):
                        nc.tensor.matmul(
                            pg[:, :w],
                            wgt[:, k, ft * 128 : (ft + 1) * 128],
                            xtile[:, k, :w],
                            start=(k == 0),
                            stop=(k == KO - 1),
                        )

                def up_mm(ft, pu):
                    for k in range(KO):
                        nc.tensor.matmul(
                            pu[:, :w],
                            wut[:, k, ft * 128 : (ft + 1) * 128],
                            xtile[:, k, :w],
                            start=(k == 0),
                            stop=(k == KO - 1),
                        )

                if j == 0:
                    # Pipeline fill: all gate matmuls first (they only need
                    # wg, the first weight DMAs to land), so the PE isn't
                    # stalled waiting for wu mid-chunk.
                    gss = []
                    for ft in range(KF):
                        pg = pg_pool.tile([128, 512], F32, tag="pg")
                        gate_mm(ft, pg)
                        gs = gpool.tile([128, 512], F32, tag="g")
                        nc.scalar.activation(gs[:, :w], pg[:, :w], SILU)
                        gss.append(gs)
                    for ft in range(KF):
                        pu = pu_pool.tile([128, 512], F32, tag="pu")
                        up_mm(ft, pu)
                        nc.vector.tensor_tensor(
                            act[:, ft, :w], gss[ft][:, :w], pu[:, :w], MULT
                        )
                else:
                    for ft in range(KF):
                        pg = pg_pool.tile([128, 512], F32, tag="pg")
                        pu = pu_pool.tile([128, 512], F32, tag="pu")
                        gate_mm(ft, pg)
                        up_mm(ft, pu)
                        gs = gpool.tile([128, 512], F32, tag="g")
                        nc.scalar.activation(gs[:, :w], pg[:, :w], SILU)
                        nc.vector.tensor_tensor(
                            act[:, ft, :w], gs[:, :w], pu[:, :w], MULT
                        )
                # x lookahead issued after the chunk's silus in Act-queue
                # order: dispatching it earlier would let its transfers jump
                # ahead of weight pieces in the DMA service order.
                issue_x(j + 2)
                if prev is not None:
                    stage_b(*prev, ydma_step=4 if j >= len(chunks) - 1 else HT)
                if new_expert and nload < len(expert_seq):
                    # Prefetch the next expert's weights AFTER the boundary
                    # chunk's y write-backs so the 3MB weight convoy doesn't
                    # jump ahead of them in the DMA service order.
                    nxt = expert_seq[nload]
                    wtiles[nxt] = load_weights(nxt)
                    nload += 1
                prev = (act, w, c0, wdt)
            stage_b(*prev, ydma_step=2)

    _split_multi_waits(nc)
    return nc, CT


_program_cache = {}


def _get_program(pads):
    key = tuple(pads)
    if key not in _program_cache:
        _program_cache[key] = _build_program(pads)
    return _program_cache[key]


def _route(x, w_gate):
    """Host router: softmax(fp32) then top-2, matching jax.lax.top_k
    tie-breaking (lowest index first)."""
    logits = x @ w_gate  # [T, E] fp32
    m = logits.max(axis=-1, keepdims=True)
    p = np.exp(logits - m, dtype=np.float32)
    p /= p.sum(axis=-1, keepdims=True)
    order = np.argsort(-p, axis=-1, kind="stable")
    sel = order[:, :TOP_K]
    rw = np.take_along_axis(p, sel, axis=-1).astype(np.float32)
    return sel, rw


def kernel(hidden_states, w_gate, w_gate_proj, w_up_proj, w_down_proj):
    import ml_dtypes

    bf16 = ml_dtypes.bfloat16
    x = np.asarray(hidden_states, dtype=np.float32).reshape(-1, H)
    w_gate = np.asarray(w_gate, dtype=np.float32)
    WG = np.asarray(w_gate_proj, dtype=np.float32)
    WU = np.asarray(w_up_proj, dtype=np.float32)
    WD = np.asarray(w_down_proj, dtype=np.float32)
    T = x.shape[0]

    sel, rw = _route(x, w_gate)

    idx, wtok, cnts = [], [], []
    for e in range(E):
        mask0 = sel[:, 0] == e
        mask1 = sel[:, 1] == e
        ie = np.nonzero(mask0 | mask1)[0]
        idx.append(ie)
        wtok.append(np.where(mask0[ie], rw[ie, 0], rw[ie, 1]).astype(np.float32))
        cnts.append(len(ie))

    # Round each expert's token count up to even (the pad column is zeros)
    # to keep every DMA/AP row 4-byte aligned in bf16.
    ecnts = [c + (c & 1) for c in cnts]
    nc, CT = _get_program(ecnts)

    base = np.concatenate([[0], np.cumsum(ecnts)])
    xt = np.zeros((H, CT), dtype=bf16)
    for e in range(E):
        if cnts[e]:
            xt[:, base[e] : base[e] + cnts[e]] = x[idx[e]].T.astype(bf16)

    in_maps = []
    for c in range(NCORES):
        in_maps.append(
            {
                "xt": xt,
                "wg": np.ascontiguousarray(WG[:, :, c * FS : (c + 1) * FS]).astype(bf16),
                "wu": np.ascontiguousarray(WU[:, :, c * FS : (c + 1) * FS]).astype(bf16),
                "wd": np.ascontiguousarray(WD[:, c * FS : (c + 1) * FS, :]).astype(bf16),
            }
        )
    global LAST_RESULTS, LAST_NC
    res = run_bass_kernel_spmd(nc, in_maps, list(range(NCORES)))
    LAST_RESULTS = res
    LAST_NC = nc

    # yp is [H, CT] (transposed bf16 partials); sum the 8 F-slices in fp32.
    ysum = res.results[0]["yp"].astype(np.float32)
    for i in range(1, NCORES):
        ysum = ysum + res.results[i]["yp"].astype(np.float32)

    out = np.zeros((T, H), dtype=np.float32)
    for e in range(E):
        if cnts[e]:
            out[idx[e]] += ysum[:, base[e] : base[e] + cnts[e]].T * wtok[e][:, None]
    return out.reshape(B, S, H).astype(np.float32)
